# revision 1
# baseline (speedup 1.0000x reference)
"""Distributed Bass attention kernel for trn2 (8 NeuronCores).

Problem: B=4,H=16,T=2048,D=128 attention w/ Q/K/V linear projections.
  qp = q@Wq.T+bq ; kp = k@Wk.T+bk ; vp = v@Wv.T+bv
  S = qp@kp.T/sqrt(128); S = where(mask==1, -1e-9, S); P=softmax(S); out = P@vp

Key identities used:
  - masked logit -1e-9 ~= 0  =>  P_unnorm[i,j] = mask? 1.0 : exp(S[i,j])
    implemented as exp(S * (1-mask)) since exp(0)=1.
  - softmax max-subtraction is unnecessary in f32 (logits ~N(0,1)).
  - out row i = (P_unnorm @ vp)[i,:] / l[i],  l[i]=sum_j P_unnorm[i,j].
    l obtained free via a ones-column appended to vp.

Sharding: 64 (b,h) slabs -> 8 per core (head/data parallel, no collectives).

Per-core dataflow (all matmuls bf16, fp32 PSUM accumulate):
  - transpose q,k,v 128x128 tiles via PE -> qT,kT,vT (bf16)
  - qpT[e,t] = WqT.T @ qT (+bq per-partition during PSUM->SBUF copy); same kpT
  - vp[t,e] natural via lhsT=vT tile, rhs=WvT; ones column appended -> vpx
  - S computed TRANSPOSED: ST[j,i] = kpT_tile.T @ qpT  (mask also transposed on
    host) so the exp output is directly P^T, the AV stationary operand.
  - O[i, 0:129] = sum_jt PT_tile.T @ vpx_tile ; col 128 = row-sum l
  - out = O[:,0:128] * (1/l) + bv
"""

import numpy as np
import ml_dtypes

import sys
sys.path.insert(0, "/opt/trn_rl_repo")

from concourse import bacc, bass, mybir
from concourse.tile import TileContext
from concourse.bass_utils import run_bass_kernel_spmd

B, H, T, D = 4, 16, 2048, 128
NCORES = 8
SLABS_PER_CORE = (B * H) // NCORES  # 8
ROWS = SLABS_PER_CORE * T  # 16384
SCALE = 1.0 / np.sqrt(D)

F32 = mybir.dt.float32
BF16 = mybir.dt.bfloat16
AF = mybir.ActivationFunctionType


def _build_nc():
    nc = bacc.Bacc(target_bir_lowering=False, trn_type="TRN2")

    q_d = nc.declare_dram_parameter("q", [ROWS, D], F32, isOutput=False)
    k_d = nc.declare_dram_parameter("k", [ROWS, D], F32, isOutput=False)
    v_d = nc.declare_dram_parameter("v", [ROWS, D], F32, isOutput=False)
    mmt_d = nc.declare_dram_parameter("mmt", [T, T], BF16, isOutput=False)
    wqt_d = nc.declare_dram_parameter("wqt", [D, D], BF16, isOutput=False)
    wkt_d = nc.declare_dram_parameter("wkt", [D, D], BF16, isOutput=False)
    wvt_d = nc.declare_dram_parameter("wvt", [D, D], BF16, isOutput=False)
    bqc_d = nc.declare_dram_parameter("bqc", [D, 1], F32, isOutput=False)
    bkc_d = nc.declare_dram_parameter("bkc", [D, 1], F32, isOutput=False)
    bvt_d = nc.declare_dram_parameter("bvt", [D, D], F32, isOutput=False)
    id_d = nc.declare_dram_parameter("ident", [D, D], F32, isOutput=False)
    out_d = nc.declare_dram_parameter("out", [ROWS, D], F32, isOutput=True)

    NT = T // 128  # 16 tiles per slab
    NC_I = T // 512  # 4 i-chunks

    with TileContext(nc) as tc:
        with (
            tc.tile_pool(name="const", bufs=1) as const_pool,
            tc.tile_pool(name="mmt", bufs=1) as mmt_pool,
            tc.tile_pool(name="qn", bufs=4) as qn_pool,
            tc.tile_pool(name="tposed", bufs=2) as tp_pool,
            tc.tile_pool(name="proj", bufs=2) as proj_pool,
            tc.tile_pool(name="pt", bufs=2) as pt_pool,
            tc.tile_pool(name="sm", bufs=3) as sm_pool,
            tc.tile_pool(name="fin", bufs=3) as fin_pool,
            tc.tile_pool(name="tr_ps", bufs=2, space="PSUM") as trps_pool,
            tc.tile_pool(name="pj_ps", bufs=2, space="PSUM") as pjps_pool,
            tc.tile_pool(name="s_ps", bufs=2, space="PSUM") as sps_pool,
            tc.tile_pool(name="o_ps", bufs=2, space="PSUM") as ops_pool,
        ):
            # ---- constants (once per core) ----
            ident = const_pool.tile([128, 128], F32, tag="ident")
            nc.sync.dma_start(out=ident[:, :], in_=id_d[:, :])
            wqt = const_pool.tile([128, 128], BF16, tag="wqt")
            nc.sync.dma_start(out=wqt[:, :], in_=wqt_d[:, :])
            wkt = const_pool.tile([128, 128], BF16, tag="wkt")
            nc.sync.dma_start(out=wkt[:, :], in_=wkt_d[:, :])
            wvt = const_pool.tile([128, 128], BF16, tag="wvt")
            nc.sync.dma_start(out=wvt[:, :], in_=wvt_d[:, :])
            bqc = const_pool.tile([128, 1], F32, tag="bqc")
            nc.sync.dma_start(out=bqc[:, :], in_=bqc_d[:, :])
            bkc = const_pool.tile([128, 1], F32, tag="bkc")
            nc.sync.dma_start(out=bkc[:, :], in_=bkc_d[:, :])
            bvt = const_pool.tile([128, 128], F32, tag="bvt")
            nc.sync.dma_start(out=bvt[:, :], in_=bvt_d[:, :])

            # transposed mask multiplier (1-mask).T, bf16, [j, i] layout
            mmt = mmt_pool.tile([128, NT * T], BF16, tag="mmt")
            for jt in range(NT):
                nc.sync.dma_start(
                    out=mmt[:, jt * T : (jt + 1) * T],
                    in_=mmt_d[jt * 128 : (jt + 1) * 128, :],
                )

            for s in range(SLABS_PER_CORE):
                r0 = s * T

                # ---- load + transpose q,k,v; project ----
                # qT/kT/vT: [128(d), T(t)] bf16
                qT = tp_pool.tile([128, T], BF16, tag="qT")
                kT = tp_pool.tile([128, T], BF16, tag="kT")
                vT = tp_pool.tile([128, T], BF16, tag="vT")
                for name, src, dstT in (("q", q_d, qT), ("k", k_d, kT), ("v", v_d, vT)):
                    for it in range(NT):
                        nat = qn_pool.tile([128, 128], F32, tag="nat")
                        nc.sync.dma_start(
                            out=nat[:, :],
                            in_=src[r0 + it * 128 : r0 + (it + 1) * 128, :],
                        )
                        tps = trps_pool.tile([128, 128], F32, tag="tr")
                        nc.tensor.transpose(tps[:, :], nat[:, :], ident[:, :])
                        # PSUM f32 -> SBUF bf16 cast; alternate engines
                        if it % 2 == 0:
                            nc.vector.tensor_copy(
                                dstT[:, it * 128 : (it + 1) * 128], tps[:, :]
                            )
                        else:
                            nc.scalar.copy(
                                dstT[:, it * 128 : (it + 1) * 128], tps[:, :]
                            )

                # qpT[e,t] / kpT[e,t] with bias added during PSUM->SBUF copy
                qpT = proj_pool.tile([128, T], BF16, tag="qpT")
                kpT = proj_pool.tile([128, T], BF16, tag="kpT")
                for srcT, w, bias, dst in ((qT, wqt, bqc, qpT), (kT, wkt, bkc, kpT)):
                    for c in range(T // 512):
                        pps = pjps_pool.tile([128, 512], F32, tag="pj")
                        nc.tensor.matmul(
                            pps[:, :],
                            w[:, :],
                            srcT[:, c * 512 : (c + 1) * 512],
                            start=True,
                            stop=True,
                        )
                        nc.scalar.activation(
                            dst[:, c * 512 : (c + 1) * 512],
                            pps[:, :],
                            AF.Identity,
                            bias=bias[:, :],
                            scale=1.0,
                        )

                # vpx: 16 blocks [128(t), 129] bf16; col 128 of each block = 1.0
                vpx = proj_pool.tile([128, NT * 129], BF16, tag="vpx")
                nc.vector.memset(vpx[:, :], 1.0)
                for jt in range(NT):
                    pps = trps_pool.tile([128, 128], F32, tag="tr")
                    nc.tensor.matmul(
                        pps[:, :],
                        vT[:, jt * 128 : (jt + 1) * 128],
                        wvt[:, :],
                        start=True,
                        stop=True,
                    )
                    nc.vector.tensor_copy(
                        vpx[:, jt * 129 : jt * 129 + 128], pps[:, :]
                    )

                # ---- attention, i-chunks of 512 ----
                for ic in range(NC_I):
                    i0 = ic * 512
                    # P^T for this i-chunk: 16 j-tiles x [128, 512] bf16
                    pt = pt_pool.tile([128, NT * 512], BF16, tag="pt")
                    for jt in range(NT):
                        sps = sps_pool.tile([128, 512], F32, tag="s")
                        nc.tensor.matmul(
                            sps[:, :],
                            kpT[:, jt * 128 : (jt + 1) * 128],
                            qpT[:, i0 : i0 + 512],
                            start=True,
                            stop=True,
                        )
                        smt = sm_pool.tile([128, 512], F32, tag="sm")
                        nc.vector.tensor_mul(
                            smt[:, :],
                            sps[:, :],
                            mmt[:, jt * T + i0 : jt * T + i0 + 512],
                        )
                        nc.scalar.activation(
                            pt[:, jt * 512 : (jt + 1) * 512],
                            smt[:, :],
                            AF.Exp,
                            scale=float(SCALE),
                        )

                    for itl in range(4):
                        io = itl * 128
                        ops = ops_pool.tile([128, 129], F32, tag="o")
                        for jt in range(NT):
                            nc.tensor.matmul(
                                ops[:, :],
                                pt[:, jt * 512 + io : jt * 512 + io + 128],
                                vpx[:, jt * 129 : (jt + 1) * 129],
                                start=(jt == 0),
                                stop=(jt == NT - 1),
                            )
                        rl = fin_pool.tile([128, 1], F32, tag="rl")
                        nc.vector.reciprocal(rl[:, :], ops[:, 128:129])
                        ot = fin_pool.tile([128, 128], F32, tag="ot")
                        nc.scalar.activation(
                            ot[:, :], ops[:, 0:128], AF.Copy, scale=rl[:, :]
                        )
                        nc.vector.tensor_add(ot[:, :], ot[:, :], bvt[:, :])
                        nc.sync.dma_start(
                            out=out_d[r0 + i0 + io : r0 + i0 + io + 128, :],
                            in_=ot[:, :],
                        )
    if not nc.is_finalized():
        nc.finalize()
    return nc


_NC_CACHE = None


def kernel(q, k, v, mask, Wq, bq, Wk, bk, Wv, bv):
    global _NC_CACHE
    if _NC_CACHE is None:
        _NC_CACHE = _build_nc()
    nc = _NC_CACHE

    bf16 = ml_dtypes.bfloat16
    qf = np.asarray(q, np.float32).reshape(B * H, T, D)
    kf = np.asarray(k, np.float32).reshape(B * H, T, D)
    vf = np.asarray(v, np.float32).reshape(B * H, T, D)
    mmt = np.ascontiguousarray(
        (1.0 - np.asarray(mask, np.float32)[0, 0]).T
    ).astype(bf16)
    wqt = np.ascontiguousarray(np.asarray(Wq, np.float32).T).astype(bf16)
    wkt = np.ascontiguousarray(np.asarray(Wk, np.float32).T).astype(bf16)
    wvt = np.ascontiguousarray(np.asarray(Wv, np.float32).T).astype(bf16)
    bqc = np.asarray(bq, np.float32).reshape(D, 1).copy()
    bkc = np.asarray(bk, np.float32).reshape(D, 1).copy()
    bvt = np.broadcast_to(np.asarray(bv, np.float32), (D, D)).copy()
    ident = np.eye(D, dtype=np.float32)

    in_maps = []
    for c in range(NCORES):
        sl = slice(c * SLABS_PER_CORE, (c + 1) * SLABS_PER_CORE)
        in_maps.append(
            {
                "q": np.ascontiguousarray(qf[sl].reshape(ROWS, D)),
                "k": np.ascontiguousarray(kf[sl].reshape(ROWS, D)),
                "v": np.ascontiguousarray(vf[sl].reshape(ROWS, D)),
                "mmt": mmt,
                "wqt": wqt,
                "wkt": wkt,
                "wvt": wvt,
                "bqc": bqc,
                "bkc": bkc,
                "bvt": bvt,
                "ident": ident,
            }
        )

    global _LAST_IN_MAPS
    _LAST_IN_MAPS = in_maps
    res = run_bass_kernel_spmd(nc, in_maps, core_ids=list(range(NCORES)))
    outs = [np.asarray(res.results[c]["out"]) for c in range(NCORES)]
    full = np.concatenate(outs, axis=0).reshape(B, H, T, D).astype(np.float32)
    return full

